# revision 33
# baseline (speedup 1.0000x reference)
"""CMC-V2 loss kernel for 8 Trainium2 NeuronCores (Bass/Tile), v4 (AllGather).

Math
----
The reference loss decomposes into:
  - 9 NT-Xent contrastive terms. For pair (A, B) with row-normalized
    embeddings Z = [An; Bn] (N=4096 rows, D=512), sim = 5*cos.  Rows are
    unit-norm so sim[i,i] = 5.0 is the exact row max:
        lse_i (diag excluded) = 5 + log(S_i - 1),  S_i = sum_j exp(5*cos_ij - 5)
    per-pair loss = 5 + (1/4096) sum_i log(S_i - 1) - (10/4096) sum_i cos_i
  - 12 cosine-embedding terms: 1 - (1/2048) sum_i cos_i.
  Total constant: 9*5 + 12 = 57.

Sharding (v4)
-------------
v1-v3 built all 12 normalized+transposed half-matrices redundantly on every
core (~3 full element passes over 12.6M elements on DVE/ScalarE per core -
the measured wall).  v4 shards the build and AllGathers the fp8 ZnT:

  - core c receives a 512-row slab: rows [256c, 256c+512) mod 2048 of each
    input (roll+slice on host), so its own 256 Gram rows are always the
    slab's rows [0:256) - statically addressed, SPMD-identical program.
  - it normalizes + PE-transposes + drains only that slab (2x redundancy
    across 8 cores instead of 8x),
  - AllGather groups [[0,2,4,6], [1,3,5,7]]: each group holds 4 distinct
    512-aligned slabs (in the group's own 256-rotated ordering - harmless,
    row sums are column-order-invariant) -> every core assembles the full
    [d, 2048] fp8 ZnT of all 12 half-matrices via DRAM bounce buffers.
  - lhsT comes from the core's private shard tile, rhs from the gathered
    ZnT.  Exp row sums / row dots as in v1; host sums 8 partial tensors.
"""

import numpy as np
import ml_dtypes
from contextlib import ExitStack

from concourse import bass, bacc, tile, mybir
from concourse.bass_utils import run_bass_kernel_spmd

BF16 = mybir.dt.bfloat16
FP8 = mybir.dt.float8e4
F32 = mybir.dt.float32
AF = mybir.ActivationFunctionType
ALU = mybir.AluOpType

USE_FP8 = True
FP8_SCALE = 16.0

B = 2048          # batch
DH = 512          # half feature dim
N_CORES = 8
R = B // N_CORES  # 256 own Gram rows per core
SLAB = 512        # rows built per core (2x redundant across 8 cores)
ST = SLAB // 128  # 4 row tiles per slab
KC = DH // 128    # 4 contraction chunks
CBW = 512         # column block width
NG = 4            # slabs per gather group

NAMES = ["f1_m0", "f1_m1", "f1_m2", "f2_m0", "f2_m1", "f2_m2"]

N_SLOTS = 9 * 4   # 9 pairs x 4 M-tiles of 128 Gram rows each
N_DOTS = 21      # 9 contrastive + 12 ortho row-dot sums


def build_program(use_fp8=USE_FP8, repeat=1, loads_on="sync",
                  psum_banks=2, psum_bufs=3, timing_mode="full",
                  drain_pat="vva", shard_bufs=12, znt_bufs=12,
                  build_stage=5, ag_group=2):
    # Restrict ACT table selection to the one set containing BOTH exp and ln.
    if not getattr(bacc, "_ant_act_tables_patched", False):
        _orig_tables = bacc.get_activation_tables

        def _patched(arch):
            tabs = _orig_tables(arch)
            return {k: (v if k == "natural_log_exp_and_others" else set())
                    for k, v in tabs.items()}

        bacc.get_activation_tables = _patched
        bacc._ant_act_tables_patched = True

    nc = bacc.Bacc(
        "TRN2",
        target_bir_lowering=False,
        debug=False,
        enable_asserts=False,
        num_devices=N_CORES,
    )
    ffs = [nc.dram_tensor(n, [SLAB, 2 * DH], BF16, kind="ExternalInput").ap()
           for n in NAMES]
    out_dram = nc.dram_tensor("part", [128, 4], F32, kind="ExternalOutput").ap()

    n_sub = 2 * B // (psum_banks * CBW)     # psum tiles per (pair, mtile)
    cb_per = psum_banks                     # 512-col blocks per psum tile

    with tile.TileContext(nc) as tc, ExitStack() as ctx:
        znt_pool = ctx.enter_context(tc.tile_pool(name="zntp", bufs=znt_bufs))
        shard_pool = ctx.enter_context(
            tc.tile_pool(name="shardp", bufs=shard_bufs))
        x_pool = ctx.enter_context(tc.tile_pool(name="xp", bufs=3))
        zn_pool = ctx.enter_context(tc.tile_pool(name="znp", bufs=4))
        vscr_pool = ctx.enter_context(tc.tile_pool(name="vscrp", bufs=3))
        escr_pool = ctx.enter_context(tc.tile_pool(name="escrp", bufs=3))
        nrm_pool = ctx.enter_context(tc.tile_pool(name="nrmp", bufs=3))
        sab_pool = ctx.enter_context(tc.tile_pool(name="sabp", bufs=4))
        acc_pool = ctx.enter_context(tc.tile_pool(name="accp", bufs=1))
        dram_pool = ctx.enter_context(
            tc.tile_pool(name="dramp", bufs=4, space="DRAM"))
        psum_pool = ctx.enter_context(
            tc.tile_pool(name="psump", bufs=psum_bufs, space="PSUM"))

        load_eng = {"gpsimd": nc.gpsimd, "scalar": nc.scalar,
                    "sync": nc.sync}[loads_on]

        biasm5 = acc_pool.tile([128, 1], F32, tag="biasm5", name="biasm5")
        nc.gpsimd.memset(biasm5[:], -5.0)
        # identity for PE transposes
        ident = acc_pool.tile([128, 128], BF16, tag="ident", name="ident")
        iota_r = acc_pool.tile([128, 128], F32, tag="iota_r", name="iota_r")
        iota_p = acc_pool.tile([128, 1], F32, tag="iota_p", name="iota_p")
        nc.gpsimd.iota(iota_r[:], pattern=[[1, 128]], base=0,
                       channel_multiplier=0,
                       allow_small_or_imprecise_dtypes=True)
        nc.gpsimd.iota(iota_p[:], pattern=[[0, 1]], base=0,
                       channel_multiplier=1,
                       allow_small_or_imprecise_dtypes=True)
        nc.vector.tensor_scalar(
            out=ident[:], in0=iota_r[:], scalar1=iota_p[:, 0:1],
            scalar2=None, op0=ALU.is_equal)
        cp_i = [0]
        sm1 = acc_pool.tile([128, N_SLOTS], F32, tag="sm1", name="sm1")
        dots_all = acc_pool.tile([128, N_DOTS], F32, tag="dots", name="dots_all")
        logv = acc_pool.tile([128, N_SLOTS], F32, tag="logv", name="logv")
        part = acc_pool.tile([128, 4], F32, tag="part", name="part_sb")

        znt = {}    # gathered [128, NG, KC, SLAB] fp8: full matrix as rhs
        shard = {}  # own slab  [128, KC, SLAB] fp8: lhsT + dots

        # Collectives cannot live inside For_i control flow (mesh desync);
        # benchmark repeats are python-unrolled instead when unroll=True.
        unroll = repeat > 1
        rep_ctx = None

        def build_ff(f):
            """Load the 512-row slab of ff tensor f; normalize both halves;
            PE-transpose + drain to the fp8 shard; AllGather the halves'
            shards into the full ZnT."""
            norms = nrm_pool.tile([128, 2 * ST], F32, tag="norms",
                                  name=f"nrm{f}")
            lgn = nrm_pool.tile([128, 2 * ST], F32, tag="lgn", name=f"lgn{f}")
            rinv = nrm_pool.tile([128, 2 * ST], F32, tag="rinv",
                                 name=f"rinv{f}")
            shs = []
            for h in range(2):
                shs.append(shard_pool.tile(
                    [128, KC, SLAB], FP8, tag="shard", name=f"sh{f}_{h}"))
            # one DMA loads the whole slab: [128, 4, 1024]
            xt = x_pool.tile([128, ST, 2 * DH], BF16, tag="xt", name=f"xt{f}")
            load_eng.dma_start(
                out=xt[:],
                in_=ffs[f][:, :].rearrange("(tt p) c -> p tt c", p=128))
            for i in range(ST):
                xv = xt[:, i, :]
                for h in range(2):
                    sq = vscr_pool.tile([128, DH], BF16, tag="sqv",
                                        name=f"sq{f}_{h}_{i}")
                    nc.vector.scalar_tensor_tensor(
                        out=sq[:], in0=xv[:, h * DH:(h + 1) * DH],
                        scalar=1.0, in1=xv[:, h * DH:(h + 1) * DH],
                        op0=ALU.mult, op1=ALU.mult,
                        accum_out=norms[:, h * ST + i:h * ST + i + 1])
            nc.scalar.activation(lgn[:], norms[:], AF.Ln)
            nc.scalar.activation(rinv[:], lgn[:], AF.Exp, scale=-0.5)
            for i in range(ST):
                xv = xt[:, i, :]
                for h in range(2):
                    zn = zn_pool.tile([128, DH], BF16, tag="zn",
                                      name=f"zn{f}_{h}_{i}")
                    nc.vector.tensor_scalar(
                        out=zn[:], in0=xv[:, h * DH:(h + 1) * DH],
                        scalar1=rinv[:, h * ST + i:h * ST + i + 1],
                        scalar2=FP8_SCALE, op0=ALU.mult, op1=ALU.mult)
                    tp = psum_pool.tile([128, KC, 128], BF16,
                                        tag="tpp", bufs=2,
                                        name=f"tp{f}_{h}_{i}")
                    for c in range(KC):
                        nc.tensor.transpose(
                            tp[:, c, :], zn[:, c * 128:(c + 1) * 128],
                            ident[:])
                    dst = shs[h][:, :, i * 128:(i + 1) * 128]
                    dch = drain_pat[cp_i[0] % len(drain_pat)]
                    if dch == "a":
                        nc.scalar.copy(dst, tp[:, :, :])
                    else:
                        nc.vector.tensor_copy(dst, tp[:, :, :])
                    cp_i[0] += 1
            for h in range(2):
                shard[(f, h)] = shs[h]

        def ag_ffs(fs):
            """One AllGather for the shards of the given ff tensors (both
            halves each): bounce to DRAM, gather, read back full ZnTs."""
            if build_stage < 5:
                return
            nsh = 2 * len(fs)
            bin_ = dram_pool.tile([128, nsh * KC * SLAB], FP8, tag="bin",
                                  name=f"bin{fs[0]}")
            bout = dram_pool.tile([128 * NG, nsh * KC * SLAB], FP8,
                                  tag="bout", name=f"bout{fs[0]}")
            for k, (f, h) in enumerate([(f, h) for f in fs for h in range(2)]):
                nc.sync.dma_start(
                    out=bin_[:, k * KC * SLAB:(k + 1) * KC * SLAB],
                    in_=shard[(f, h)][:].rearrange("p c j -> p (c j)"))
            nc.gpsimd.collective_compute(
                "AllGather", ALU.bypass,
                replica_groups=[[0, 2, 4, 6], [1, 3, 5, 7]],
                ins=[bin_.opt()], outs=[bout.opt()])
            # gathered flat layout: [rank][p][(sh c j)]
            for k, (f, h) in enumerate([(f, h) for f in fs for h in range(2)]):
                zt = znt_pool.tile([128, NG, KC, SLAB], FP8, tag="znt",
                                   name=f"znt{f}_{h}")
                nc.sync.dma_start(
                    out=zt[:],
                    in_=bout[:, k * KC * SLAB:(k + 1) * KC * SLAB].rearrange(
                        "(r p) (c j) -> p r c j", r=NG, c=KC))
                znt[(f, h)] = zt

        slot_i = [0]

        def gram(A, Bm):
            """Gram rows + fused exp/rowsum for contrastive pair (A, Bm).
            lhsT from the core's own shard (its rows are the slab's first
            256), rhs from the gathered ZnT: 2 matrices x 4 slabs x 512
            (group-rotated column order - irrelevant to row sums)."""
            for X in (A, Bm):          # lhsT source matrix
                for mt in range(2):    # two 128-row M tiles of own rows
                    si = slot_i[0]
                    sab = sab_pool.tile([128, n_sub], F32, tag="sab",
                                        name=f"sab{si}")
                    for ridx, RH in enumerate((A, Bm)):   # rhs matrix
                        for sub in range(n_sub // 2):
                            ps = psum_pool.tile(
                                [128, cb_per, CBW], F32, tag="gram",
                                name=f"ps{si}_{ridx}_{sub}")
                            for cbl in range(cb_per):
                                r4 = sub * cb_per + cbl   # slab index
                                for q in range(KC // 2):
                                    nc.tensor.matmul(
                                        ps[:, cbl, :],
                                        shard[X][:, 2 * q:2 * q + 2,
                                                 mt * 128:(mt + 1) * 128],
                                        znt[RH][:, r4, 2 * q:2 * q + 2, :],
                                        perf_mode=mybir.MatmulPerfMode.DoubleRow,
                                        start=(q == 0),
                                        stop=(q == KC // 2 - 1))
                            es = escr_pool.tile([128, cb_per, CBW], BF16,
                                                tag="escr",
                                                name=f"es{si}_{ridx}_{sub}")
                            col = ridx * (n_sub // 2) + sub
                            nc.scalar.activation(
                                es[:], ps[:], AF.Exp, bias=biasm5[:],
                                scale=5.0 / (FP8_SCALE * FP8_SCALE),
                                accum_out=sab[:, col:col + 1])
                    scr2 = sab_pool.tile([128, n_sub], F32, tag="scr2",
                                         name=f"scr2_{si}")
                    nc.vector.tensor_scalar(
                        out=scr2[:], in0=sab[:], scalar1=-1.0 / n_sub,
                        scalar2=None, op0=ALU.add, op1=ALU.add,
                        accum_out=sm1[:, si:si + 1])
                    slot_i[0] += 1

        def dots(col, X, Y):
            """dots_all[:, col] = per-partition sum over the core's own 256
            rows of <Zn_X[i], Zn_Y[i]>."""
            o = vscr_pool.tile([128, KC, R], F32, tag="vscr", name=f"do{col}")
            nc.vector.scalar_tensor_tensor(
                out=o[:], in0=shard[X][:, :, 0:R],
                scalar=1.0 / (FP8_SCALE * FP8_SCALE),
                in1=shard[Y][:, :, 0:R], op0=ALU.mult, op1=ALU.mult,
                accum_out=dots_all[:, col:col + 1])

        if timing_mode == "grams":
            sh_shared, zt_shared = [], []
            for s in range(9):
                t = znt_pool.tile([128, NG, KC, SLAB], FP8, tag="znt",
                                  name=f"znts{s}")
                nc.vector.memset(t[:, 0, 0, 0:2], 0.0)
                zt_shared.append(t)
                u = shard_pool.tile([128, KC, SLAB], FP8, tag="shard",
                                    name=f"shs{s}")
                nc.vector.memset(u[:, 0, 0:2], 0.0)
                sh_shared.append(u)
            for f in range(6):
                for h in range(2):
                    znt[(f, h)] = zt_shared[(2 * f + h) % 9]
                    shard[(f, h)] = sh_shared[(2 * f + h) % 9]
            def build_ff(f):
                pass
            def ag_ffs(fs):
                pass
        elif timing_mode == "builds":
            nc.vector.memset(sm1[:], 1.0)
            nc.vector.memset(dots_all[:], 0.0)
            def gram(A, Bm):
                pass
            def dots(col, X, Y):
                pass

        for _rep in range(repeat):
            slot_i[0] = 0
            cp_i[0] = 0
            build_ff(0)
            build_ff(1)
            ag_ffs([0, 1])
            build_ff(2)
            ag_ffs([2])
            gram((0, 0), (1, 0))
            dots(0, (0, 0), (1, 0))
            dots(9, (0, 0), (0, 1)); dots(10, (1, 0), (1, 1))
            dots(12, (0, 1), (1, 1))
            build_ff(3)
            ag_ffs([3])
            gram((0, 0), (2, 0)); gram((1, 0), (2, 0))
            dots(1, (0, 0), (2, 0)); dots(2, (1, 0), (2, 0))
            dots(11, (2, 0), (2, 1))
            dots(13, (0, 1), (2, 1)); dots(14, (1, 1), (2, 1))
            build_ff(4)
            ag_ffs([4])
            gram((0, 1), (3, 1))
            dots(3, (0, 1), (3, 1))
            dots(15, (3, 0), (3, 1))
            build_ff(5)
            ag_ffs([5])
            gram((1, 1), (4, 1)); gram((3, 0), (4, 0))
            dots(4, (1, 1), (4, 1)); dots(6, (3, 0), (4, 0))
            dots(16, (4, 0), (4, 1)); dots(18, (3, 1), (4, 1))
            gram((2, 1), (5, 1)); gram((3, 0), (5, 0)); gram((4, 0), (5, 0))
            dots(5, (2, 1), (5, 1)); dots(7, (3, 0), (5, 0))
            dots(8, (4, 0), (5, 0))
            dots(17, (5, 0), (5, 1)); dots(19, (3, 1), (5, 1))
            dots(20, (4, 1), (5, 1))

        # ---- epilogue ----
        nc.scalar.activation(logv[:], sm1[:], AF.Ln)
        nc.vector.memset(part[:], 0.0)
        nc.vector.tensor_reduce(part[:, 0:1], logv[:], axis=mybir.AxisListType.X,
                                op=ALU.add)
        nc.vector.tensor_reduce(part[:, 1:2], dots_all[:, 0:9],
                                axis=mybir.AxisListType.X, op=ALU.add)
        nc.vector.tensor_reduce(part[:, 2:3], dots_all[:, 9:21],
                                axis=mybir.AxisListType.X, op=ALU.add)
        nc.sync.dma_start(out=out_dram, in_=part[:])

    nc.compile()
    return nc


_PROG = None


def _get_prog():
    global _PROG
    if _PROG is None:
        _PROG = build_program()
    return _PROG


def make_in_maps(inputs):
    bf = ml_dtypes.bfloat16
    in_maps = []
    for c in range(N_CORES):
        m = {}
        for n in NAMES:
            a = np.asarray(inputs[n], dtype=np.float32)
            sl = np.take(a, range(R * c, R * c + SLAB), axis=0, mode="wrap")
            m[n] = np.ascontiguousarray(sl).astype(bf)
        in_maps.append(m)
    return in_maps


def combine(parts):
    """parts: list of 8 [128, 4] f32 arrays -> scalar loss."""
    tl = tcc = toc = 0.0
    for p in parts:
        p = np.asarray(p, dtype=np.float64)
        tl += p[:, 0].sum()
        tcc += p[:, 1].sum()
        toc += p[:, 2].sum()
    n2 = float(2 * B)
    loss = (9 * 5.0 + 12.0) + tl / n2 - 10.0 * tcc / n2 - toc / float(B)
    return np.float32(loss)


def kernel(**inputs):
    nc = _get_prog()
    in_maps = make_in_maps(inputs)
    res = run_bass_kernel_spmd(nc, in_maps, list(range(N_CORES)))
    return combine([res.results[c]["part"] for c in range(N_CORES)])
